# revision 11
# baseline (speedup 1.0000x reference)
"""CharRNN forward Trainium2 kernel (8 NeuronCores, data-parallel over batch).

Reference computation:
    x_embed = embedding[x]                       # [b, l, e]
    xw = einsum('ble,eh->lbh', x_embed, W_xh) + b_h
    h_{t} = tanh(xw_{t-1} + h_{t-1} @ W_hh)      # scan over l
    logits = outs @ W_hy_w.T + W_hy_b            # [b, l, v]
    returns (logits, final_hidden)

Device strategy (per core, batch shard of 8 rows):
  * Fold embedding+input projection into a single table E' = emb @ W_xh + b_h
    (computed on device, [256, 512]).
  * Keep the recurrent state TRANSPOSED (hT, [512, 8] as 4x[128, 8] chunks).
    Per step: 16 matmuls out[j,b] += W_hh[k,j]^T-chunk @ hT-chunk with W_hh
    chunks as the stationary operand (bf16 -> fast weight load), accumulating
    into a PSUM bank that was PRE-FILLED with xw^T for a 16-step window by
    one-hot @ E' matmuls (has_written set by PE, so start=False accumulates).
    tanh on ScalarE reads the bank slice and writes the next hT chunk set
    directly -- no per-step transposes, no vector-add on the critical path.
  * Logits tiles [128 rows = 16 steps x 8 batch, 256 vocab] from outsT
    (stationary) x W_hy^T (moving), bias added on VectorE, DMA'd out.
"""

import sys

if "/opt/trn_rl_repo" not in sys.path:
    sys.path.insert(0, "/opt/trn_rl_repo")

from contextlib import ExitStack

import ml_dtypes
import numpy as np

import concourse.bass as bass
import concourse.tile as tile
from concourse import mybir
from concourse.bass_utils import run_bass_kernel_spmd

BATCH = 64
SEQ = 1024
VOCAB = 256
EMBED = 128
HIDDEN = 512
NCORES = 8
BPC = BATCH // NCORES  # batch rows per core
WIN = 16  # recurrence steps per PSUM window (16*8 = 128 columns)

BF16 = mybir.dt.bfloat16
F32 = mybir.dt.float32
F32R = mybir.dt.float32r

_BUILD_CACHE: dict = {}
LAST_RUN_INFO: dict = {}


def _split_multi_waits(nc: bass.Bass) -> int:
    """Walrus on this stack encodes at most ONE semaphore wait per engine
    instruction. Split extra waits onto preceding single-wait EventSemaphore
    nops on the same (FIFO) engine queue -- semantics preserved."""
    fn = nc.m.functions[0]
    n_split = 0
    for blk in fn.blocks:
        new_insts = []
        for inst in blk.instructions:
            si = inst.sync_info
            if si is not None and si.on_wait and len(si.on_wait) > 1:
                waits = list(si.on_wait)
                for k, wsub in enumerate(waits[:-1]):
                    n_split += 1
                    new_insts.append(
                        mybir.InstEventSemaphore(
                            name=f"{inst.name}-wsplit{k}",
                            engine=inst.engine,
                            ins=[],
                            outs=[],
                            sync_info=mybir.SyncInfo(on_wait=[wsub], on_update=[]),
                        )
                    )
                inst.sync_info = mybir.SyncInfo(
                    on_wait=[waits[-1]], on_update=list(si.on_update)
                )
            new_insts.append(inst)
        blk.instructions = new_insts
    return n_split


def build_nc(seq: int = SEQ) -> bass.Bass:
    """Build the single-core (SPMD) bass program for a batch shard of BPC."""
    assert seq % WIN == 0
    nwin = seq // WIN
    nc = bass.Bass()

    x_d = nc.declare_dram_parameter("x", [1, seq * BPC], F32, isOutput=False)
    h0_d = nc.declare_dram_parameter("h0", [4, 128, BPC], BF16, isOutput=False)
    embT_d = nc.declare_dram_parameter("embT", [EMBED, VOCAB], BF16, isOutput=False)
    wxh_d = nc.declare_dram_parameter("wxh", [EMBED, HIDDEN], BF16, isOutput=False)
    bh_d = nc.declare_dram_parameter("bh", [1, HIDDEN], F32, isOutput=False)
    whh_d = nc.declare_dram_parameter("whh", [4, 128, HIDDEN], BF16, isOutput=False)
    whyT_d = nc.declare_dram_parameter("whyT", [4, 128, VOCAB], BF16, isOutput=False)
    bhy_d = nc.declare_dram_parameter("bhy", [1, VOCAB], F32, isOutput=False)
    iota_d = nc.declare_dram_parameter("iota2", [128, 2], F32, isOutput=False)
    ident_d = nc.declare_dram_parameter("ident", [128, 128], BF16, isOutput=False)

    # t-major layout: row (t*BPC + b) -> host un-permutes to [BPC, seq, VOCAB]
    logits_d = nc.declare_dram_parameter("logits", [seq * BPC, VOCAB], F32, isOutput=True)
    hout_d = nc.declare_dram_parameter("hout", [BPC, HIDDEN], F32, isOutput=True)

    with tile.TileContext(nc) as tc, ExitStack() as ctx:
        const = ctx.enter_context(tc.tile_pool(name="const", bufs=1))
        ohp = ctx.enter_context(tc.tile_pool(name="ohp", bufs=2))
        lsbp = ctx.enter_context(tc.tile_pool(name="lsbp", bufs=3))
        ps_xw = ctx.enter_context(tc.tile_pool(name="ps_xw", bufs=2, space="PSUM"))
        ps_l = ctx.enter_context(tc.tile_pool(name="ps_l", bufs=2, space="PSUM"))
        ps_m = ctx.enter_context(tc.tile_pool(name="ps_m", bufs=2, space="PSUM"))

        # ---- constants / parameters into SBUF ----
        embT_sb = const.tile([128, VOCAB], BF16)
        nc.gpsimd.dma_start(out=embT_sb, in_=embT_d[:, :])
        wxh_sb = const.tile([128, HIDDEN], BF16)
        nc.gpsimd.dma_start(out=wxh_sb, in_=wxh_d[:, :])
        bh_rep = const.tile([128, HIDDEN], F32)
        nc.gpsimd.dma_start(out=bh_rep, in_=bh_d[:, :].to_broadcast((128, HIDDEN)))
        W_sb = const.tile([128, 4, HIDDEN], BF16)
        nc.gpsimd.dma_start(out=W_sb, in_=whh_d[:].rearrange("k p j -> p k j"))
        WyT_sb = const.tile([128, 4, VOCAB], BF16)
        nc.gpsimd.dma_start(out=WyT_sb, in_=whyT_d[:].rearrange("k p v -> p k v"))
        by_rep = const.tile([128, VOCAB], F32)
        nc.gpsimd.dma_start(out=by_rep, in_=bhy_d[:, :].to_broadcast((128, VOCAB)))
        iota_sb = const.tile([128, 2], F32)
        nc.gpsimd.dma_start(out=iota_sb, in_=iota_d[:, :])
        ident_sb = const.tile([128, 128], BF16)
        nc.gpsimd.dma_start(out=ident_sb, in_=ident_d[:, :])

        # x broadcast across partitions: [128, seq*BPC] fp32 (token ids)
        x_rep = const.tile([128, seq * BPC], F32)
        ncol = seq * BPC
        chunk = 1024
        for c0 in range(0, ncol, chunk):
            c1 = min(c0 + chunk, ncol)
            nc.gpsimd.dma_start(
                out=x_rep[:, c0:c1],
                in_=x_d[:, c0:c1].to_broadcast((128, c1 - c0)),
            )

        # outsT[p, kc, tau, b] = hT[kc*128+p, b] after step tau (tau=0 -> h0)
        outsT = const.tile([128, 4, seq + 1, BPC], BF16)
        nc.gpsimd.dma_start(
            out=outsT[:, :, 0, :], in_=h0_d[:].rearrange("k p b -> p k b")
        )

        # ---- E' = emb @ W_xh + b_h, stored bf16 as lhsT chunks ----
        E_sb = const.tile([128, 2, HIDDEN], BF16)
        for vc in range(2):
            ps_e = ps_m.tile([128, HIDDEN], F32, tag="ps_e")
            nc.tensor.matmul(
                out=ps_e,
                lhsT=embT_sb[:, vc * 128 : (vc + 1) * 128],
                rhs=wxh_sb,
                start=True,
                stop=True,
            )
            nc.vector.tensor_add(E_sb[:, vc, :], ps_e, bh_rep)

        def emit_oh(w):
            """Build one-hot window tile for window w (steps w*WIN..w*WIN+WIN-1)."""
            oh = ohp.tile([128, 2, WIN * BPC], BF16, tag="oh")
            for vc in range(2):
                nc.vector.tensor_scalar(
                    out=oh[:, vc, :],
                    in0=x_rep[:, w * WIN * BPC : (w + 1) * WIN * BPC],
                    scalar1=iota_sb[:, vc : vc + 1],
                    scalar2=None,
                    op0=mybir.AluOpType.is_equal,
                )
            return oh

        def emit_xw_mm(xwb, oh, jc, vc, first):
            # first xw matmul opens the bank's accumulation group (clears
            # has_written for the whole bank); the last one closes the sim-side
            # group so ScalarE may read the bank. The recurrence matmuls then
            # accumulate with skip_group_check (has_written bits persist).
            nc.tensor.matmul(
                out=xwb[:, jc, :],
                lhsT=E_sb[:, vc, jc * 128 : (jc + 1) * 128],
                rhs=oh[:, vc, :],
                start=first,
                stop=(jc == 3 and vc == 1),
            )

        def emit_logits(w, lmm_only=None):
            """Logits for window w. Called in two phases: matmuls, then tail."""
            pass

        # Window-state carried across the emission loop
        oh_cur = emit_oh(0)
        xwb_cur = ps_xw.tile([128, 4, WIN * BPC], F32, tag="xwb")
        for jc in range(4):
            for vc in range(2):
                emit_xw_mm(xwb_cur, oh_cur, jc, vc, first=(jc == 0 and vc == 0))

        lg_psum_prev = None  # psum tile holding logits matmul accum of window w-1
        lg_win_prev = None

        for w in range(nwin):
            # Prepare next window's one-hot early (DVE is idle)
            if w + 1 < nwin:
                oh_next = emit_oh(w + 1)
                xwb_next = ps_xw.tile([128, 4, WIN * BPC], F32, tag="xwb")
            else:
                oh_next = None
                xwb_next = None

            xw_mm_list = (
                [(jc, vc) for jc in range(4) for vc in range(2)] if oh_next else []
            )
            # logits matmuls for the PREVIOUS window get distributed into this
            # window's ScalarE-wait gaps (tl = 0..3)
            lg_psum = None

            for tl in range(WIN):
                tau = w * WIN + tl + 1
                # distribute next-window xw matmuls into gaps (tl 4..11)
                if oh_next and 4 <= tl < 12:
                    jc, vc = xw_mm_list[tl - 4]
                    emit_xw_mm(xwb_next, oh_next, jc, vc, first=(tl == 4))
                # distribute previous-window logits matmuls (tl 0..3)
                if lg_psum_prev is not None and tl < 4:
                    hc = tl
                    nc.tensor.matmul(
                        out=lg_psum_prev,
                        lhsT=outsT[
                            :, hc, lg_win_prev * WIN + 1 : (lg_win_prev + 1) * WIN + 1, :
                        ],
                        rhs=WyT_sb[:, hc, :],
                        start=(hc == 0),
                        stop=(hc == 3),
                    )

                # recurrence: 16 matmuls accumulate onto xw in the bank
                for jc in range(4):
                    for kc in range(4):
                        nc.tensor.matmul(
                            out=xwb_cur[:, jc, tl * BPC : (tl + 1) * BPC],
                            lhsT=W_sb[:, kc, jc * 128 : (jc + 1) * 128],
                            rhs=outsT[:, kc, tau - 1, :],
                            start=False,
                            stop=False,
                            skip_group_check=True,
                        )
                nc.scalar.activation(
                    out=outsT[:, :, tau, :],
                    in_=xwb_cur[:, :, tl * BPC : (tl + 1) * BPC],
                    func=mybir.ActivationFunctionType.Tanh,
                )
                # finish previous window's logits after its matmuls are done
                if lg_psum_prev is not None and tl == 4:
                    lsb = lsbp.tile([128, VOCAB], F32, tag="lsb")
                    nc.vector.tensor_add(lsb, lg_psum_prev, by_rep)
                    nc.sync.dma_start(
                        out=logits_d[
                            lg_win_prev * WIN * BPC : (lg_win_prev + 1) * WIN * BPC, :
                        ],
                        in_=lsb,
                    )
                    lg_psum_prev = None

            # queue this window's logits work (runs during next window)
            lg_psum_prev = ps_l.tile([128, VOCAB], F32, tag="lg")
            lg_win_prev = w
            oh_cur = oh_next
            xwb_cur = xwb_next

        # final window's logits (no next window to hide them in)
        for hc in range(4):
            nc.tensor.matmul(
                out=lg_psum_prev,
                lhsT=outsT[
                    :, hc, lg_win_prev * WIN + 1 : (lg_win_prev + 1) * WIN + 1, :
                ],
                rhs=WyT_sb[:, hc, :],
                start=(hc == 0),
                stop=(hc == 3),
            )
        lsb = lsbp.tile([128, VOCAB], F32, tag="lsb")
        nc.vector.tensor_add(lsb, lg_psum_prev, by_rep)
        nc.sync.dma_start(
            out=logits_d[
                lg_win_prev * WIN * BPC : (lg_win_prev + 1) * WIN * BPC, :
            ],
            in_=lsb,
        )

        # final hidden: transpose hT[seq] -> [BPC, HIDDEN] fp32
        ps_h = ps_m.tile([BPC, HIDDEN], BF16, tag="ps_h")
        for kc in range(4):
            nc.tensor.matmul(
                out=ps_h[:, kc * 128 : (kc + 1) * 128],
                lhsT=outsT[:, kc, seq, :],
                rhs=ident_sb,
                is_transpose=True,
                start=(kc == 0),
                stop=(kc == 3),
            )
        h_sb = lsbp.tile([BPC, HIDDEN], F32, tag="hsb")
        nc.vector.tensor_copy(h_sb, ps_h)
        nc.sync.dma_start(out=hout_d[:, :], in_=h_sb)

    return nc


def make_in_maps(
    x: np.ndarray,
    hidden: np.ndarray,
    embedding: np.ndarray,
    W_xh: np.ndarray,
    W_hh: np.ndarray,
    b_h: np.ndarray,
    W_hy_w: np.ndarray,
    W_hy_b: np.ndarray,
    seq: int,
):
    bf = ml_dtypes.bfloat16
    embT = np.ascontiguousarray(np.asarray(embedding, np.float32).T).astype(bf)  # [128, 256]
    wxh = np.ascontiguousarray(np.asarray(W_xh, np.float32)).astype(bf)  # [128, 512]
    bh = np.asarray(b_h, np.float32).reshape(1, HIDDEN)
    whh = np.ascontiguousarray(
        np.asarray(W_hh, np.float32).reshape(4, 128, HIDDEN).astype(bf)
    )
    whyT = np.ascontiguousarray(
        np.asarray(W_hy_w, np.float32).T.reshape(4, 128, VOCAB).astype(bf)
    )
    bhy = np.asarray(W_hy_b, np.float32).reshape(1, VOCAB)
    iota2 = np.ascontiguousarray(
        np.arange(256, dtype=np.float32).reshape(2, 128).T
    )  # [128, 2]
    ident = np.eye(128, dtype=np.float32).astype(bf)

    in_maps = []
    for c in range(NCORES):
        xs = np.asarray(x[c * BPC : (c + 1) * BPC, :seq], np.float32)  # [BPC, seq]
        xs = np.ascontiguousarray(xs.T).reshape(1, seq * BPC)  # t-major
        hs = np.asarray(hidden[c * BPC : (c + 1) * BPC], np.float32)  # [BPC, 512]
        h0 = np.ascontiguousarray(hs.T.reshape(4, 128, BPC)).astype(bf)
        in_maps.append(
            {
                "x": xs,
                "h0": h0,
                "embT": embT,
                "wxh": wxh,
                "bh": bh,
                "whh": whh,
                "whyT": whyT,
                "bhy": bhy,
                "iota2": iota2,
                "ident": ident,
            }
        )
    return in_maps


def kernel(x, hidden, embedding, W_xh, W_hh, b_h, W_hy_w, W_hy_b):
    global LAST_RUN_INFO
    seq = int(np.asarray(x).shape[1])
    if seq not in _BUILD_CACHE:
        nc_new = build_nc(seq)
        # CoreSim can't execute bare EventSemaphore waits, so the wait-split
        # legalization happens only on the hardware path.
        _split_multi_waits(nc_new)
        _BUILD_CACHE[seq] = nc_new
    nc = _BUILD_CACHE[seq]
    in_maps = make_in_maps(
        x, hidden, embedding, W_xh, W_hh, b_h, W_hy_w, W_hy_b, seq
    )
    res = run_bass_kernel_spmd(nc, in_maps, list(range(NCORES)))
    LAST_RUN_INFO = {
        "exec_time_ns": res.exec_time_ns,
        "mean_exec_time_ns": res.mean_exec_time_ns,
    }
    logits = np.concatenate(
        [
            res.results[c]["logits"].reshape(seq, BPC, VOCAB).transpose(1, 0, 2)
            for c in range(NCORES)
        ],
        axis=0,
    )
    final_hidden = np.concatenate(
        [res.results[c]["hout"] for c in range(NCORES)], axis=0
    )
    return logits, final_hidden


if __name__ == "__main__":
    # quick smoke test with random data
    rng = np.random.default_rng(0)
    seq = int(sys.argv[1]) if len(sys.argv) > 1 else 64
    x = rng.integers(0, VOCAB, (BATCH, seq))
    hidden = np.zeros((BATCH, HIDDEN), np.float32)
    emb = rng.standard_normal((VOCAB, EMBED), np.float32)
    W_xh = (rng.standard_normal((EMBED, HIDDEN)) * 0.01).astype(np.float32)
    W_hh = (rng.standard_normal((HIDDEN, HIDDEN)) * 0.01).astype(np.float32)
    b_h = np.zeros((HIDDEN,), np.float32)
    W_hy_w = (rng.standard_normal((VOCAB, HIDDEN)) / np.sqrt(HIDDEN)).astype(
        np.float32
    )
    W_hy_b = (rng.standard_normal((VOCAB,)) * 0.01).astype(np.float32)

    logits, fh = kernel(x, hidden, emb, W_xh, W_hh, b_h, W_hy_w, W_hy_b)

    # numpy reference
    xw = emb[x] @ W_xh + b_h  # [b, l, h]
    h = hidden.copy()
    outs = np.zeros((BATCH, seq, HIDDEN), np.float32)
    for t in range(seq):
        h = np.tanh(xw[:, t, :] + h @ W_hh)
        outs[:, t, :] = h
    ref_logits = outs @ W_hy_w.T + W_hy_b
    el = np.max(np.abs(logits - ref_logits)) / (np.max(np.abs(ref_logits)) + 1e-30)
    eh = np.max(np.abs(fh - h)) / (np.max(np.abs(h)) + 1e-30)
    print("logits relerr:", el, "hidden relerr:", eh)
    print("run info:", LAST_RUN_INFO)
